# revision 69
# baseline (speedup 1.0000x reference)
"""Bass/Trainium2 kernel for nn_BasicBlock_73933567033945 (CDConv / gnn_message_passing).

v2 strategy (graph = fixed +-8 sequence window inside 4 chains, verified at
runtime): shard 8192 nodes across 8 cores (1024 each, half a chain), slot
layout of 128-row halo windows at stride 112.  All matmuls and DVE tensor ops
run in fp16 (fp32 PSUM accumulation); pos is slot-centered on host so fp16
holds precision.  The 17 window shifts are materialized once per core by 17
wide shift-matmuls over all 10 slots (h|pos|ori, 440 cols each).  The
per-edge kernel MLP output is written pair-duplicated (kern2) so the
bilinear kern (x) h product runs in the DVE 2x perf mode.  The (offset,
channel) contraction runs on the PE via PSUM-accumulated transposes followed
by Wk-chunk matmuls, all fp16.  Pure data parallel: no collectives.
"""
import numpy as np

B, L, C = 4, 2048, 128
N = B * L
W = 32
KC = 24
SEQ_L = 11
R = 12.0
WIN = 8
NEG_IN = 0.1
NEG_K = 0.2
NCORES = 8
NPC = N // NCORES          # 1024 nodes per core
TS = 112                   # output nodes per tile
NT = 10                    # tiles per core (9*112 + 16)
HR = 9 * TS + 128          # 1136 halo rows per core
K17 = 2 * WIN + 1          # 17 window offsets
S_HALF = SEQ_L // 2
PH = 44                    # phys cols per slot: h(32) | pos(3) | ori(9)
NBW = NT * PH              # 440: NB cols per k

_PROG = {}


def _sidx(k):
    return int(np.clip(k - WIN, -S_HALF, S_HALF)) + S_HALF


def _build_program():
    import concourse.tile as tile
    from concourse import mybir, bacc
    from concourse.bass_utils import run_bass_kernel_spmd  # noqa: F401 (import check)
    from contextlib import ExitStack

    f32 = mybir.dt.float32
    f16 = mybir.dt.float16
    AF = mybir.ActivationFunctionType
    OP = mybir.AluOpType
    AX = mybir.AxisListType

    nc = bacc.Bacc("TRN2", target_bir_lowering=False, debug=False)

    def din(name, shape, dt=f16):
        return nc.dram_tensor(name, shape, dt, kind="ExternalInput").ap()

    xT_slot = din("xT_slot", [128, NT * 128])        # x transposed per slot, f16
    xc_slot = din("xc_slot", [128, NT * C])          # identity (center rows) f16
    pog_slot = din("pog_slot", [128, NT * 12])       # centered pos(3) | ori(9)
    w_in = din("w_in", [C, W])
    ws2a = din("ws2a", [128, 2 * K17 * KC])
    ws2b = din("ws2b", [8, 2 * K17 * KC])
    wk_p = din("wk_p", [128, 6 * W])
    w_out = din("w_out", [W, C])
    ident = din("ident", [128, 128])
    shifts = din("shifts", [128, K17 * TS])
    shiftsd = din("shiftsd", [128, K17 * TS])
    kself2 = din("kself2", [128, NT * 2 * KC])
    y = nc.dram_tensor("y", [NPC, C], f32, kind="ExternalOutput").ap()

    P = TS  # 112 active partitions

    with tile.TileContext(nc) as tc, ExitStack() as ctx:
        pers = ctx.enter_context(tc.tile_pool(name="pers", bufs=1))

        def load(ap_in, shape, tag, dt=f16):
            t = pers.tile(shape, dt, tag=tag)
            nc.sync.dma_start(t[:], ap_in)
            return t

        # DMA order matters: pos/ori + shift matrices + identity feed the
        # NBg matmuls that start first; xc is only needed at tile ends.
        phys_g = pers.tile([128, NT * 12], f16, tag="phys_g")
        nc.sync.dma_start(phys_g[:], pog_slot)
        sh_sb = pers.tile([128, K17 * TS], f16, tag="shifts")
        for c0 in range(0, K17 * TS, 5 * TS):
            c1 = min(c0 + 5 * TS, K17 * TS)
            nc.sync.dma_start(sh_sb[:, c0:c1], shifts[:, c0:c1])
        shd_sb = load(shiftsd, [128, K17 * TS], "shiftsd")
        id_sb = load(ident, [128, 128], "ident")
        w_in_sb = load(w_in, [C, W], "w_in")
        xT_all = pers.tile([128, NT * 128], f16, tag="xT_all")
        nc.sync.dma_start(xT_all[:, 0:5 * 128], xT_slot[:, 0:5 * 128])
        nc.sync.dma_start(xT_all[:, 5 * 128:], xT_slot[:, 5 * 128:])
        ws2a_sb = load(ws2a, [128, 2 * K17 * KC], "ws2a")
        ws2b_sb = load(ws2b, [8, 2 * K17 * KC], "ws2b")
        wk_sb = load(wk_p, [128, 6 * W], "wk")
        w_out_sb = load(w_out, [W, C], "w_out")
        ks2_sb = load(kself2, [128, NT * 2 * KC], "kself2")
        xc_all = load(xc_slot, [128, NT * C], "xc_all")

        # dist = sqrt(d2 + eps): eps = 1e-4 keeps rec = 1/dist <= 100 (fp16
        # safe; self-edges have D = 0 so local = 0 regardless) while real
        # edge distances (>= ~0.5) are perturbed by < 1e-3 relative.
        eps_sb = pers.tile([128, 1], f32, tag="eps")
        nc.vector.memset(eps_sb[:], 1e-4)
        phys_h = pers.tile([128, NT * W], f16, tag="phys_h")

        # ---- NBg: pos/ori shift matmuls (independent of Phase A) ---------
        # pos uses (S_k - I) as the stationary so the result is directly
        # D_k = pos[p+k] - pos[p]; ori uses plain S_k.  phys_g layout is
        # [pos slots (30) | ori slots (90)]; NBg k-block is [D (30) | ori (90)].
        NBg = pers.tile([P, K17 * NT * 12], f16, tag="NBg")
        NBh = pers.tile([P, K17 * NT * W], f16, tag="NBh")
        GW = NT * 12   # 120
        HW_ = NT * W   # 320
        with tc.tile_pool(name="pNBg", bufs=3, space="PSUM") as pNBg:
            for k in range(K17):
                nb_p = pNBg.tile([P, GW], f32, tag="nbg_p")
                nc.tensor.matmul(nb_p[:, 0:30], shd_sb[:, TS * k:TS * (k + 1)],
                                 phys_g[:, 0:30], start=True, stop=False,
                                 skip_group_check=True)
                nc.tensor.matmul(nb_p[:, 30:GW], sh_sb[:, TS * k:TS * (k + 1)],
                                 phys_g[:, 30:GW], start=False, stop=True,
                                 skip_group_check=True)
                dst = NBg[:, GW * k:GW * (k + 1)]
                if k % 2 == 0:
                    nc.vector.tensor_copy(dst, nb_p[:])
                else:
                    nc.scalar.copy(dst, nb_p[:])

        # ---------------- Phase A: h = lrelu(lrelu(x) @ W_in) per slot -----
        with tc.tile_pool(name="pA", bufs=2) as pA, \
             tc.tile_pool(name="pAp", bufs=2, space="PSUM") as pAp:
            for j in range(NT):
                xlT = pA.tile([128, 128], f16, tag="xlT")
                nc.scalar.activation(xlT[:], xT_all[:, 128 * j:128 * (j + 1)],
                                     AF.Prelu, bias=0.0, scale=1.0, alpha=NEG_IN)
                hp = pAp.tile([128, W], f32, tag="hp")
                nc.tensor.matmul(hp[:], xlT[:], w_in_sb[:], start=True, stop=True)
                nc.scalar.activation(phys_h[:, W * j:W * (j + 1)], hp[:],
                                     AF.Prelu, bias=0.0, scale=1.0, alpha=NEG_IN)

        # ---- NBh: h shift matmuls, split by slot halves so early tiles
        # unblock as soon as the first half of Phase A lands --------------
        HH = 5 * W  # 160
        with tc.tile_pool(name="pNBh", bufs=3, space="PSUM") as pNBh:
            for half in range(2):
                for k in range(K17):
                    nb_p = pNBh.tile([P, HH], f32, tag="nbh_p")
                    nc.tensor.matmul(nb_p[:], sh_sb[:, TS * k:TS * (k + 1)],
                                     phys_h[:, HH * half:HH * (half + 1)],
                                     start=True, stop=True)
                    dst = NBh[:, HW_ * k + HH * half:HW_ * k + HH * (half + 1)]
                    if k % 2 == 0:
                        nc.scalar.copy(dst, nb_p[:])
                    else:
                        nc.vector.tensor_copy(dst, nb_p[:])

        # ---------------- Phase B: per output tile ------------------------
        wrk = ctx.enter_context(tc.tile_pool(name="wrk", bufs=4))
        tpool = ctx.enter_context(tc.tile_pool(name="tmp", bufs=8))
        psG = ctx.enter_context(tc.tile_pool(name="psG", bufs=2, space="PSUM"))
        psD = ctx.enter_context(tc.tile_pool(name="psD", bufs=1, space="PSUM"))
        psP = ctx.enter_context(tc.tile_pool(name="psP", bufs=1, space="PSUM"))
        psC = ctx.enter_context(tc.tile_pool(name="psC", bufs=1, space="PSUM"))

        for t in range(NT):
            # k-strided views into NBg for slot t: D block at 3t, ori at 30+9t
            def kview(off, width):
                v = NBg[:].rearrange("p (k j) -> p k j", j=GW)
                return v[:, :, off:off + width]

            ori_c = NBg[:, GW * 8 + 30 + 9 * t:GW * 8 + 30 + 9 * t + 9]

            # ---- geometry -> dav [P, (k,8)] fp16 -------------------------
            # D = pos[p+k] - pos[p] comes straight from the (S_k - I) shifts
            Dv = kview(3 * t, 3)
            sq = wrk.tile([P, K17 * 3], f16, tag="sq")
            sqv = sq[:].rearrange("p (k a) -> p k a", a=3)
            nc.vector.tensor_mul(sqv, Dv, Dv)
            d2 = wrk.tile([P, K17], f32, tag="d2")
            nc.vector.tensor_reduce(d2[:], sq[:].rearrange("p (k a) -> p k a", a=3),
                                    axis=AX.X, op=OP.add)
            dav = wrk.tile([P, K17 * 8], f16, tag="dav")
            davv = dav[:].rearrange("p (k d) -> p k d", d=8)
            # dist/R into delta slot 6 (sqrt(d2)/R)
            nc.scalar.activation(davv[:, :, 6], d2[:], AF.Sqrt, bias=0.0,
                                 scale=1.0 / (R * R))
            dist = wrk.tile([P, K17], f32, tag="dist")
            nc.scalar.activation(dist[:], d2[:], AF.Sqrt, bias=eps_sb[0:P, 0:1],
                                 scale=1.0)
            rec = wrk.tile([P, K17], f16, tag="rec")
            with nc.allow_low_precision(reason="fp16 direction scale is ok"):
                nc.vector.reciprocal(rec[:], dist[:])
            # local_a = (sum_b Ri[a,b] * D[k,b]) * rec[k]
            lm = wrk.tile([P, K17 * 9], f16, tag="lm")
            lmv = lm[:].rearrange("p (k a b) -> p k a b", a=3, b=3)
            nc.vector.tensor_mul(
                lmv,
                ori_c.rearrange("p (a b) -> p a b", b=3).unsqueeze(1)
                     .broadcast_to([P, K17, 3, 3]),
                Dv.unsqueeze(2).broadcast_to([P, K17, 3, 3]))
            locr = wrk.tile([P, K17 * 3], f16, tag="locr")
            with nc.allow_low_precision(reason="3-term sums, fp16 ok"):
                nc.vector.tensor_reduce(
                    locr[:].rearrange("p (k a) -> p k a", a=3), lmv,
                    axis=AX.X, op=OP.add)
            nc.vector.tensor_mul(
                davv[:, :, 0:3], locr[:].rearrange("p (k a) -> p k a", a=3),
                rec[:].unsqueeze(-1).broadcast_to([P, K17, 3]))
            # ofeat_a = sum_b Ri[a,b] * Rj[a,b]
            ofm = wrk.tile([P, K17 * 9], f16, tag="ofm")
            nc.vector.tensor_mul(
                ofm[:].rearrange("p (k e) -> p k e", e=9), kview(30 + 9 * t, 9),
                ori_c.unsqueeze(1).broadcast_to([P, K17, 9]))
            with nc.allow_low_precision(reason="3-term sums, fp16 ok"):
                nc.vector.tensor_reduce(
                    davv[:, :, 3:6],
                    ofm[:].rearrange("p (k a b) -> p k a b", a=3, b=3),
                    axis=AX.X, op=OP.add)
            # out-of-chain halo rows are zero-padded on host, so h_src = 0
            # there and fake-edge messages vanish without any masking.  The
            # dav bias slot (7) is only ever written here, so set it just
            # once per pool buffer rotation.
            if t < 4:
                nc.vector.memset(davv[:, :, 7], 1.0)

            # ---- kern2 = lrelu(dav @ WS2, 0.2), pair-duplicated ----------
            dT_p = psD.tile([128, 224], f16, tag="dT")
            nc.tensor.matmul(dT_p[:, 0:P], dav[:, 0:128], id_sb[0:P, 0:P],
                             is_transpose=True, start=True, stop=False,
                             skip_group_check=True)
            nc.tensor.matmul(dT_p[0:8, P:P + P], dav[:, 128:136], id_sb[0:P, 0:P],
                             is_transpose=True, start=False, stop=True,
                             skip_group_check=True)
            dT = wrk.tile([128, 224], f16, tag="dT_sb")
            nc.scalar.copy(dT[:], dT_p[:])
            W2 = 2 * K17 * KC  # 816
            # psum banks are 512 f32 cols: put k-blocks 0..9 at 0:480 (bank 0)
            # and k-blocks 10..16 at 512:848 (bank 1) to avoid bank crossing.
            pre_p = psP.tile([P, 848], f32, tag="pre")
            nc.tensor.matmul(pre_p[:, 0:480], dT[:, 0:P], ws2a_sb[:, 0:480],
                             start=True, stop=False, skip_group_check=True)
            nc.tensor.matmul(pre_p[:, 512:848], dT[:, 0:P], ws2a_sb[:, 480:W2],
                             start=True, stop=False, skip_group_check=True)
            nc.tensor.matmul(pre_p[:, 0:480], dT[0:8, P:P + P], ws2b_sb[:, 0:480],
                             start=False, stop=True, skip_group_check=True)
            nc.tensor.matmul(pre_p[:, 512:848], dT[0:8, P:P + P], ws2b_sb[:, 480:W2],
                             start=False, stop=True, skip_group_check=True)
            kern2 = wrk.tile([P, W2], f16, tag="kern2")
            nc.scalar.activation(kern2[:, 0:480], pre_p[:, 0:480], AF.Prelu,
                                 bias=0.0, scale=1.0, alpha=NEG_K)
            nc.scalar.activation(kern2[:, 480:W2], pre_p[:, 512:848], AF.Prelu,
                                 bias=0.0, scale=1.0, alpha=NEG_K)
            # self-edge compensation (host-precomputed, pair-duplicated);
            # nonzero only at chain ends, which land in tiles 0 and 9
            if t in (0, NT - 1):
                K8 = 2 * KC * 8
                nc.vector.tensor_add(kern2[:, K8:K8 + 2 * KC],
                                     kern2[:, K8:K8 + 2 * KC],
                                     ks2_sb[0:P, 2 * KC * t:2 * KC * (t + 1)])

            # ---- bilinear + PE transpose-accumulate ----------------------
            # gpsimd takes the last 3 offsets (issued first so they finish
            # by the time the PE transpose chain reaches them); DVE does the
            # rest in the 2x packed mode.
            # sum_k kern_k (x) h_k accumulated in normal layout via
            # identity-stationary copy-matmuls (2 per k, split at the psum
            # bank boundary), then ONE transpose set of 6 matmuls.  The
            # same psum tile is reused for the transposed result after the
            # accumulation has been copied out (WAR dep keeps it safe).
            ag_ps = psG.tile([128, 768], f32, tag="agg")
            agg_p = ag_ps[0:P, :]
            for k in range(K17):
                tm = tpool.tile([P, KC * W], f16, tag="tm")
                hv = NBh[:, HW_ * k + W * t:HW_ * k + W * (t + 1)] \
                    .rearrange("p (s two) -> p s two", two=2) \
                    .unsqueeze(1).broadcast_to([P, KC, 16, 2])
                kv = kern2[:, 2 * KC * k:2 * KC * (k + 1)] \
                    .rearrange("p (c two) -> p c two", two=2) \
                    .unsqueeze(2).broadcast_to([P, KC, 16, 2])
                nc.vector.tensor_tensor(
                    tm[:].rearrange("p (c s two) -> p c s two", two=2, s=16),
                    hv, kv, op=OP.mult)
                nc.tensor.matmul(agg_p[:, 0:512], id_sb[0:P, 0:P],
                                 tm[:, 0:512], start=(k == 0), stop=(k == 16),
                                 skip_group_check=True)
                nc.tensor.matmul(agg_p[:, 512:768], id_sb[0:P, 0:P],
                                 tm[:, 512:768], start=(k == 0), stop=(k == 16),
                                 skip_group_check=True)
            agg = wrk.tile([P, 768], f16, tag="agg_sb")
            nc.scalar.copy(agg[:], agg_p[:])
            aggT_p = ag_ps
            for b in range(6):
                nc.tensor.matmul(aggT_p[:, 128 * b:128 * b + P],
                                 agg[:, 128 * b:128 * (b + 1)], id_sb[0:P, 0:P],
                                 start=(b in (0, 4)), stop=(b in (3, 5)),
                                 skip_group_check=True)
            aggT = wrk.tile([128, 768], f16, tag="aggT_sb")
            nc.scalar.copy(aggT[:], aggT_p[:])

            # ---- conv = lrelu(agg @ Wk, 0.1) ; out = conv @ W_out + x ----
            co_p = psC.tile([P, 240], f32, tag="co")
            for b in range(6):
                nc.tensor.matmul(co_p[0:W, 0:P], wk_sb[:, W * b:W * (b + 1)],
                                 aggT[:, 128 * b:128 * b + P],
                                 start=(b == 0), stop=(b == 5),
                                 skip_group_check=True)
            convL = wrk.tile([W, P], f16, tag="convL")
            nc.scalar.activation(convL[:], co_p[0:W, 0:P], AF.Prelu, bias=0.0,
                                 scale=1.0, alpha=NEG_IN)
            nc.tensor.matmul(co_p[:, P:P + 128], convL[:], w_out_sb[:],
                             start=True, stop=False, skip_group_check=True)
            # identity add on the PE: accumulate xc into the same psum group
            # via an identity-stationary copy-matmul, then DMA from PSUM.
            nc.tensor.matmul(co_p[:, P:P + 128], id_sb[0:P, 0:P],
                             xc_all[0:P, C * t:C * t + C],
                             start=False, stop=True, skip_group_check=True)
            out_sb = wrk.tile([P, C], f32, tag="out_sb")
            nc.scalar.copy(out_sb[:], co_p[:, P:P + 128])
            cnt = min(TS, NPC - TS * t)
            nc.sync.dma_start(y[TS * t:TS * t + cnt, :], out_sb[0:cnt, :])

    nc.compile()
    return nc


def _expected_src_dst():
    i = np.arange(N)
    offs = np.arange(-WIN, WIN + 1)
    j = i[:, None] + offs[None, :]
    valid = ((j // L) == (i[:, None] // L)) & (j >= 0) & (j < N)
    j = np.where(valid, j, i[:, None])
    dst = np.repeat(i, offs.size).astype(np.int32)
    src = j.reshape(-1).astype(np.int32)
    return src, dst


def _host_inputs(x, pos, ori, W_in, Ws0, bs0, Wk, W_out):
    xf = np.ascontiguousarray(x.reshape(N, C), np.float32)
    pos = np.asarray(pos, np.float32)
    ori = np.asarray(ori, np.float32)
    f16 = np.float16

    # shared weights / constants
    WS = np.zeros((136, K17 * KC), np.float32)
    for k in range(K17):
        s = _sidx(k)
        WS[8 * k:8 * k + 7, KC * k:KC * (k + 1)] = Ws0[s]
        WS[8 * k + 7, KC * k:KC * (k + 1)] = bs0[s]
    # pair-duplicate columns: WS2[:, 48k + 2c + j] = WS[:, 24k + c]
    WS2 = np.repeat(WS, 2, axis=1)
    wk_p = np.zeros((128, 6 * W), np.float32)
    for b in range(6):
        wk_p[:, W * b:W * (b + 1)] = Wk[128 * b:128 * (b + 1), :]
    shifts = np.zeros((128, K17 * TS), np.float32)
    shiftsd = np.zeros((128, K17 * TS), np.float32)
    for k in range(K17):
        for p in range(TS):
            shifts[p + k, TS * k + p] = 1.0
            shiftsd[p + k, TS * k + p] += 1.0
            shiftsd[p + 8, TS * k + p] -= 1.0   # minus center (k=8 row)
    common = dict(
        w_in=W_in.astype(f16),
        ws2a=WS2[0:128].astype(f16),
        ws2b=WS2[128:136].astype(f16),
        wk_p=wk_p.astype(f16),
        w_out=W_out.astype(f16),
        ident=np.eye(128, dtype=f16),
        shifts=shifts.astype(f16),
        shiftsd=shiftsd.astype(f16),
    )

    # self-edge compensation: kself[n] = lrelu(rn @ W5[3:6] + b5, 0.2) * ncl
    rn = (ori.reshape(N, 3, 3) ** 2).sum(axis=2)          # [N, 3]
    pself = rn @ np.asarray(Ws0[S_HALF][3:6], np.float32) \
        + np.asarray(bs0[S_HALF], np.float32)             # [N, KC]
    kself_full = np.where(pself >= 0, pself, NEG_K * pself)

    in_maps = []
    for ci in range(NCORES):
        s0 = ci * NPC
        g = s0 - WIN + np.arange(HR)
        # chain-aware zero padding: out-of-chain halo rows get h = 0, so
        # their messages vanish with no explicit masking on device.
        ok = (g // L) == (s0 // L)
        gi = np.clip(g, 0, N - 1)
        x_pad = np.where(ok[:, None], xf[gi], 0.0).astype(np.float32)
        p_pad = np.where(ok[:, None], pos[gi], 0.0).astype(np.float32)
        o_pad = np.where(ok[:, None], ori[gi], 0.0).astype(np.float32)

        jj, pp = np.meshgrid(np.arange(NT), np.arange(128), indexing="ij")
        rows = (TS * jj + pp)            # [NT, 128] all < HR
        # xT_slot: [128(c), (t, p)] transposed slots
        x_sl = x_pad[rows]               # [NT, 128, C]
        xT_slot = np.ascontiguousarray(
            x_sl.transpose(2, 0, 1).reshape(C, NT * 128)).astype(f16)
        # pos: center per slot for fp16 precision; block layout [pos | ori]
        p_sl = p_pad[rows]               # [NT, 128, 3]
        ctr = p_sl.mean(axis=1, keepdims=True)
        pos_b = (p_sl - ctr).transpose(1, 0, 2).reshape(128, NT * 3)
        ori_b = o_pad[rows].transpose(1, 0, 2).reshape(128, NT * 9)
        pog_slot = np.ascontiguousarray(
            np.concatenate([pos_b, ori_b], axis=1)).astype(f16)
        # identity (center rows)
        rc = WIN + TS * jj + pp
        okc = rc < HR
        xc_slot = np.where(okc[:, :, None], x_pad[np.minimum(rc, HR - 1)], 0.0)
        xc_slot = xc_slot.transpose(1, 0, 2).reshape(128, NT * C).astype(f16)

        # boundary-count + kself2 (output-node indexed)
        ncl = np.zeros((128, NT), np.float32)
        for t in (0, NT - 1):
            nvalid = min(TS, NPC - TS * t)
            for p in range(nvalid):
                off = (s0 + TS * t + p) % L
                v = ((off + np.arange(-WIN, WIN + 1)) >= 0) & \
                    ((off + np.arange(-WIN, WIN + 1)) < L)
                ncl[p, t] = K17 - v.sum()
        ks = np.zeros((128, NT, KC), np.float32)
        for t in (0, NT - 1):
            nvalid = min(TS, NPC - TS * t)
            rowsn = s0 + TS * t + np.arange(nvalid)
            ks[:nvalid, t, :] = kself_full[rowsn] * ncl[:nvalid, t][:, None]
        ks2 = np.repeat(ks, 2, axis=2)  # duplicate pairs within each KC block
        in_maps.append(dict(
            xT_slot=xT_slot, xc_slot=xc_slot, pog_slot=pog_slot,
            kself2=ks2.reshape(128, NT * 2 * KC).astype(f16),
            **common))
    return in_maps


def kernel(x, pos, seq, ori, W_in, Ws0, bs0, Wk, W_out, src, dst):
    exp_src, exp_dst = _expected_src_dst()
    assert np.array_equal(np.asarray(src), exp_src), "unexpected src graph"
    assert np.array_equal(np.asarray(dst), exp_dst), "unexpected dst graph"

    from concourse.bass_utils import run_bass_kernel_spmd

    if "nc" not in _PROG:
        _PROG["nc"] = _build_program()
    nc = _PROG["nc"]

    in_maps = _host_inputs(np.asarray(x), np.asarray(pos), np.asarray(ori),
                           np.asarray(W_in), np.asarray(Ws0), np.asarray(bs0),
                           np.asarray(Wk), np.asarray(W_out))
    res = run_bass_kernel_spmd(nc, in_maps, list(range(NCORES)))
    out = np.concatenate([res.results[i]["y"] for i in range(NCORES)], axis=0)
    return out.reshape(B, L, C).astype(np.float32)


# revision 74
# speedup vs baseline: 1.0194x; 1.0194x over previous
"""Bass/Trainium2 kernel for nn_BasicBlock_73933567033945 (CDConv / gnn_message_passing).

v2 strategy (graph = fixed +-8 sequence window inside 4 chains, verified at
runtime): shard 8192 nodes across 8 cores (1024 each, half a chain), slot
layout of 128-row halo windows at stride 112.  All matmuls and DVE tensor ops
run in fp16 (fp32 PSUM accumulation); pos is slot-centered on host so fp16
holds precision.  The 17 window shifts are materialized once per core by 17
wide shift-matmuls over all 10 slots (h|pos|ori, 440 cols each).  The
per-edge kernel MLP output is written pair-duplicated (kern2) so the
bilinear kern (x) h product runs in the DVE 2x perf mode.  The (offset,
channel) contraction runs on the PE via PSUM-accumulated transposes followed
by Wk-chunk matmuls, all fp16.  Pure data parallel: no collectives.
"""
import numpy as np

B, L, C = 4, 2048, 128
N = B * L
W = 32
KC = 24
SEQ_L = 11
R = 12.0
WIN = 8
NEG_IN = 0.1
NEG_K = 0.2
NCORES = 8
NPC = N // NCORES          # 1024 nodes per core
TS = 112                   # output nodes per tile
NT = 10                    # tiles per core (9*112 + 16)
HR = 9 * TS + 128          # 1136 halo rows per core
K17 = 2 * WIN + 1          # 17 window offsets
S_HALF = SEQ_L // 2
PH = 44                    # phys cols per slot: h(32) | pos(3) | ori(9)
NBW = NT * PH              # 440: NB cols per k

_PROG = {}


def _sidx(k):
    return int(np.clip(k - WIN, -S_HALF, S_HALF)) + S_HALF


def _build_program():
    import concourse.tile as tile
    from concourse import mybir, bacc
    from concourse.bass_utils import run_bass_kernel_spmd  # noqa: F401 (import check)
    from contextlib import ExitStack

    f32 = mybir.dt.float32
    f16 = mybir.dt.float16
    AF = mybir.ActivationFunctionType
    OP = mybir.AluOpType
    AX = mybir.AxisListType

    nc = bacc.Bacc("TRN2", target_bir_lowering=False, debug=False)

    def din(name, shape, dt=f16):
        return nc.dram_tensor(name, shape, dt, kind="ExternalInput").ap()

    xT_slot = din("xT_slot", [128, NT * 128])        # x transposed per slot, f16
    xc_slot = din("xc_slot", [128, NT * C])          # identity (center rows) f16
    pog_slot = din("pog_slot", [128, NT * 12])       # centered pos(3) | ori(9)
    w_in = din("w_in", [C, W])
    ws2a = din("ws2a", [128, 2 * K17 * KC])
    ws2b = din("ws2b", [8, 2 * K17 * KC])
    wk_p = din("wk_p", [128, 6 * W])
    w_out = din("w_out", [W, C])
    ident = din("ident", [128, 128])
    shifts = din("shifts", [128, K17 * TS])
    kself2 = din("kself2", [128, NT * 2 * KC])
    y = nc.dram_tensor("y", [NPC, C], f32, kind="ExternalOutput").ap()

    P = TS  # 112 active partitions

    with tile.TileContext(nc) as tc, ExitStack() as ctx:
        pers = ctx.enter_context(tc.tile_pool(name="pers", bufs=1))

        def load(ap_in, shape, tag, dt=f16):
            t = pers.tile(shape, dt, tag=tag)
            nc.sync.dma_start(t[:], ap_in)
            return t

        # DMA order matters: pos/ori + shift matrices + identity feed the
        # NBg matmuls that start first; xc is only needed at tile ends.
        phys_g = pers.tile([128, NT * 12], f16, tag="phys_g")
        nc.sync.dma_start(phys_g[:], pog_slot)
        sh_sb = pers.tile([128, K17 * TS], f16, tag="shifts")
        for c0 in range(0, K17 * TS, 5 * TS):
            c1 = min(c0 + 5 * TS, K17 * TS)
            nc.sync.dma_start(sh_sb[:, c0:c1], shifts[:, c0:c1])
        id_sb = load(ident, [128, 128], "ident")
        w_in_sb = load(w_in, [C, W], "w_in")
        xT_all = pers.tile([128, NT * 128], f16, tag="xT_all")
        nc.sync.dma_start(xT_all[:, 0:5 * 128], xT_slot[:, 0:5 * 128])
        nc.sync.dma_start(xT_all[:, 5 * 128:], xT_slot[:, 5 * 128:])
        ws2a_sb = load(ws2a, [128, 2 * K17 * KC], "ws2a")
        ws2b_sb = load(ws2b, [8, 2 * K17 * KC], "ws2b")
        wk_sb = load(wk_p, [128, 6 * W], "wk")
        w_out_sb = load(w_out, [W, C], "w_out")
        ks2_sb = load(kself2, [128, NT * 2 * KC], "kself2")
        xc_all = load(xc_slot, [128, NT * C], "xc_all")

        # dist = sqrt(d2 + eps): eps = 1e-4 keeps rec = 1/dist <= 100 (fp16
        # safe; self-edges have D = 0 so local = 0 regardless) while real
        # edge distances (>= ~0.5) are perturbed by < 1e-3 relative.
        eps_sb = pers.tile([128, 1], f32, tag="eps")
        nc.vector.memset(eps_sb[:], 1e-4)
        phys_h = pers.tile([128, NT * W], f16, tag="phys_h")

        # ---- NBg: 17 pos/ori shift matmuls (independent of Phase A) ------
        NBg = pers.tile([P, K17 * NT * 12], f16, tag="NBg")
        NBh = pers.tile([P, K17 * NT * W], f16, tag="NBh")
        GW = NT * 12   # 120
        HW_ = NT * W   # 320
        with tc.tile_pool(name="pNBg", bufs=3, space="PSUM") as pNBg:
            for k in range(K17):
                nb_p = pNBg.tile([P, GW], f32, tag="nbg_p")
                nc.tensor.matmul(nb_p[:], sh_sb[:, TS * k:TS * (k + 1)],
                                 phys_g[:], start=True, stop=True)
                dst = NBg[:, GW * k:GW * (k + 1)]
                if k % 2 == 0:
                    nc.vector.tensor_copy(dst, nb_p[:])
                else:
                    nc.scalar.copy(dst, nb_p[:])

        # ---------------- Phase A: h = lrelu(lrelu(x) @ W_in) per slot -----
        with tc.tile_pool(name="pA", bufs=2) as pA, \
             tc.tile_pool(name="pAp", bufs=2, space="PSUM") as pAp:
            for j in range(NT):
                xlT = pA.tile([128, 128], f16, tag="xlT")
                nc.scalar.activation(xlT[:], xT_all[:, 128 * j:128 * (j + 1)],
                                     AF.Prelu, bias=0.0, scale=1.0, alpha=NEG_IN)
                hp = pAp.tile([128, W], f32, tag="hp")
                nc.tensor.matmul(hp[:], xlT[:], w_in_sb[:], start=True, stop=True)
                nc.scalar.activation(phys_h[:, W * j:W * (j + 1)], hp[:],
                                     AF.Prelu, bias=0.0, scale=1.0, alpha=NEG_IN)

        # ---- NBh: 17 h shift matmuls ------------------------------------
        with tc.tile_pool(name="pNBh", bufs=3, space="PSUM") as pNBh:
            for k in range(K17):
                nb_p = pNBh.tile([P, HW_], f32, tag="nbh_p")
                nc.tensor.matmul(nb_p[:], sh_sb[:, TS * k:TS * (k + 1)],
                                 phys_h[:], start=True, stop=True)
                dst = NBh[:, HW_ * k:HW_ * (k + 1)]
                if k % 2 == 0:
                    nc.scalar.copy(dst, nb_p[:])
                else:
                    nc.vector.tensor_copy(dst, nb_p[:])

        # ---------------- Phase B: per output tile ------------------------
        wrk = ctx.enter_context(tc.tile_pool(name="wrk", bufs=4))
        tpool = ctx.enter_context(tc.tile_pool(name="tmp", bufs=8))
        psG = ctx.enter_context(tc.tile_pool(name="psG", bufs=2, space="PSUM"))
        psD = ctx.enter_context(tc.tile_pool(name="psD", bufs=1, space="PSUM"))
        psP = ctx.enter_context(tc.tile_pool(name="psP", bufs=1, space="PSUM"))
        psC = ctx.enter_context(tc.tile_pool(name="psC", bufs=1, space="PSUM"))

        for t in range(NT):
            # k-strided views into NBg for slot t (pos at off 0, ori at 3)
            def kview(off, width):
                v = NBg[:].rearrange("p (k j) -> p k j", j=GW)
                return v[:, :, 12 * t + off:12 * t + off + width]

            gc = GW * 8 + 12 * t
            pos_c = NBg[:, gc:gc + 3]       # [P, 3] center pos
            ori_c = NBg[:, gc + 3:gc + 12]  # [P, 9] center frame

            # ---- geometry -> dav [P, (k,8)] fp16 -------------------------
            D = wrk.tile([P, K17 * 3], f16, tag="D")
            Dv = D[:].rearrange("p (k a) -> p k a", a=3)
            nc.vector.tensor_sub(Dv, kview(0, 3),
                                 pos_c.unsqueeze(1).broadcast_to([P, K17, 3]))
            sq = wrk.tile([P, K17 * 3], f16, tag="sq")
            nc.scalar.activation(sq[:], D[:], AF.Square, bias=0.0, scale=1.0)
            d2 = wrk.tile([P, K17], f32, tag="d2")
            nc.vector.tensor_reduce(d2[:], sq[:].rearrange("p (k a) -> p k a", a=3),
                                    axis=AX.X, op=OP.add)
            dav = wrk.tile([P, K17 * 8], f16, tag="dav")
            davv = dav[:].rearrange("p (k d) -> p k d", d=8)
            # delta slot 6 holds raw dist (WS2 rows for slot 6 are pre-scaled
            # by 1/R on host); the same values feed the reciprocal.
            nc.scalar.activation(davv[:, :, 6], d2[:], AF.Sqrt,
                                 bias=eps_sb[0:P, 0:1], scale=1.0)
            rec = wrk.tile([P, K17], f16, tag="rec")
            with nc.allow_low_precision(reason="fp16 direction scale is ok"):
                nc.vector.reciprocal(rec[:], davv[:, :, 6])
            # local_a = (sum_b Ri[a,b] * D[k,b]) * rec[k]
            lm = wrk.tile([P, K17 * 9], f16, tag="lm")
            lmv = lm[:].rearrange("p (k a b) -> p k a b", a=3, b=3)
            nc.vector.tensor_mul(
                lmv,
                ori_c.rearrange("p (a b) -> p a b", b=3).unsqueeze(1)
                     .broadcast_to([P, K17, 3, 3]),
                D[:].rearrange("p (k b) -> p k b", b=3).unsqueeze(2)
                    .broadcast_to([P, K17, 3, 3]))
            locr = wrk.tile([P, K17 * 3], f16, tag="locr")
            with nc.allow_low_precision(reason="3-term sums, fp16 ok"):
                nc.vector.tensor_reduce(
                    locr[:].rearrange("p (k a) -> p k a", a=3), lmv,
                    axis=AX.X, op=OP.add)
            nc.vector.tensor_mul(
                davv[:, :, 0:3], locr[:].rearrange("p (k a) -> p k a", a=3),
                rec[:].unsqueeze(-1).broadcast_to([P, K17, 3]))
            # ofeat_a = sum_b Ri[a,b] * Rj[a,b]
            ofm = wrk.tile([P, K17 * 9], f16, tag="ofm")
            nc.vector.tensor_mul(
                ofm[:].rearrange("p (k e) -> p k e", e=9), kview(3, 9),
                ori_c.unsqueeze(1).broadcast_to([P, K17, 9]))
            with nc.allow_low_precision(reason="3-term sums, fp16 ok"):
                nc.vector.tensor_reduce(
                    davv[:, :, 3:6],
                    ofm[:].rearrange("p (k a b) -> p k a b", a=3, b=3),
                    axis=AX.X, op=OP.add)
            # out-of-chain halo rows are zero-padded on host, so h_src = 0
            # there and fake-edge messages vanish without any masking.  The
            # dav bias slot (7) is only ever written here, so set it just
            # once per pool buffer rotation.
            if t < 4:
                nc.vector.memset(davv[:, :, 7], 1.0)

            # ---- kern2 = lrelu(dav @ WS2, 0.2), pair-duplicated ----------
            dT_p = psD.tile([128, 224], f16, tag="dT")
            nc.tensor.matmul(dT_p[:, 0:P], dav[:, 0:128], id_sb[0:P, 0:P],
                             is_transpose=True, start=True, stop=False,
                             skip_group_check=True)
            nc.tensor.matmul(dT_p[0:8, P:P + P], dav[:, 128:136], id_sb[0:P, 0:P],
                             is_transpose=True, start=False, stop=True,
                             skip_group_check=True)
            dT = wrk.tile([128, 224], f16, tag="dT_sb")
            nc.scalar.copy(dT[:], dT_p[:])
            W2 = 2 * K17 * KC  # 816
            # psum banks are 512 f32 cols: put k-blocks 0..9 at 0:480 (bank 0)
            # and k-blocks 10..16 at 512:848 (bank 1) to avoid bank crossing.
            pre_p = psP.tile([P, 848], f32, tag="pre")
            nc.tensor.matmul(pre_p[:, 0:480], dT[:, 0:P], ws2a_sb[:, 0:480],
                             start=True, stop=False, skip_group_check=True)
            nc.tensor.matmul(pre_p[:, 512:848], dT[:, 0:P], ws2a_sb[:, 480:W2],
                             start=True, stop=False, skip_group_check=True)
            nc.tensor.matmul(pre_p[:, 0:480], dT[0:8, P:P + P], ws2b_sb[:, 0:480],
                             start=False, stop=True, skip_group_check=True)
            nc.tensor.matmul(pre_p[:, 512:848], dT[0:8, P:P + P], ws2b_sb[:, 480:W2],
                             start=False, stop=True, skip_group_check=True)
            kern2 = wrk.tile([P, W2], f16, tag="kern2")
            nc.scalar.activation(kern2[:, 0:480], pre_p[:, 0:480], AF.Prelu,
                                 bias=0.0, scale=1.0, alpha=NEG_K)
            nc.scalar.activation(kern2[:, 480:W2], pre_p[:, 512:848], AF.Prelu,
                                 bias=0.0, scale=1.0, alpha=NEG_K)
            # self-edge compensation (host-precomputed, pair-duplicated);
            # nonzero only at chain ends, which land in tiles 0 and 9
            if t in (0, NT - 1):
                K8 = 2 * KC * 8
                nc.vector.tensor_add(kern2[:, K8:K8 + 2 * KC],
                                     kern2[:, K8:K8 + 2 * KC],
                                     ks2_sb[0:P, 2 * KC * t:2 * KC * (t + 1)])

            # ---- bilinear + PE transpose-accumulate ----------------------
            # gpsimd takes the last 3 offsets (issued first so they finish
            # by the time the PE transpose chain reaches them); DVE does the
            # rest in the 2x packed mode.
            # sum_k kern_k (x) h_k accumulated in normal layout via
            # identity-stationary copy-matmuls (2 per k, split at the psum
            # bank boundary), then ONE transpose set of 6 matmuls.  The
            # same psum tile is reused for the transposed result after the
            # accumulation has been copied out (WAR dep keeps it safe).
            ag_ps = psG.tile([128, 768], f32, tag="agg")
            agg_p = ag_ps[0:P, :]
            for k in range(K17):
                tm = tpool.tile([P, KC * W], f16, tag="tm")
                hv = NBh[:, HW_ * k + W * t:HW_ * k + W * (t + 1)] \
                    .rearrange("p (s two) -> p s two", two=2) \
                    .unsqueeze(1).broadcast_to([P, KC, 16, 2])
                kv = kern2[:, 2 * KC * k:2 * KC * (k + 1)] \
                    .rearrange("p (c two) -> p c two", two=2) \
                    .unsqueeze(2).broadcast_to([P, KC, 16, 2])
                nc.vector.tensor_tensor(
                    tm[:].rearrange("p (c s two) -> p c s two", two=2, s=16),
                    hv, kv, op=OP.mult)
                nc.tensor.matmul(agg_p[:, 0:512], id_sb[0:P, 0:P],
                                 tm[:, 0:512], start=(k == 0), stop=(k == 16),
                                 skip_group_check=True)
                nc.tensor.matmul(agg_p[:, 512:768], id_sb[0:P, 0:P],
                                 tm[:, 512:768], start=(k == 0), stop=(k == 16),
                                 skip_group_check=True)
            agg = wrk.tile([P, 768], f16, tag="agg_sb")
            nc.scalar.copy(agg[:], agg_p[:])
            aggT_p = ag_ps
            for b in range(6):
                nc.tensor.matmul(aggT_p[:, 128 * b:128 * b + P],
                                 agg[:, 128 * b:128 * (b + 1)], id_sb[0:P, 0:P],
                                 start=(b in (0, 4)), stop=(b in (3, 5)),
                                 skip_group_check=True)
            aggT = wrk.tile([128, 768], f16, tag="aggT_sb")
            nc.scalar.copy(aggT[:, 0:512], aggT_p[:, 0:512])
            nc.scalar.copy(aggT[:, 512:768], aggT_p[:, 512:768])

            # ---- conv = lrelu(agg @ Wk, 0.1) ; out = conv @ W_out + x ----
            co_p = psC.tile([P, 240], f32, tag="co")
            for b in range(6):
                nc.tensor.matmul(co_p[0:W, 0:P], wk_sb[:, W * b:W * (b + 1)],
                                 aggT[:, 128 * b:128 * b + P],
                                 start=(b == 0), stop=(b == 5),
                                 skip_group_check=True)
            convL = wrk.tile([W, P], f16, tag="convL")
            nc.scalar.activation(convL[:], co_p[0:W, 0:P], AF.Prelu, bias=0.0,
                                 scale=1.0, alpha=NEG_IN)
            nc.tensor.matmul(co_p[:, P:P + 128], convL[:], w_out_sb[:],
                             start=True, stop=False, skip_group_check=True)
            # identity add on the PE: accumulate xc into the same psum group
            # via an identity-stationary copy-matmul, then DMA from PSUM.
            nc.tensor.matmul(co_p[:, P:P + 128], id_sb[0:P, 0:P],
                             xc_all[0:P, C * t:C * t + C],
                             start=False, stop=True, skip_group_check=True)
            out_sb = wrk.tile([P, C], f32, tag="out_sb")
            nc.scalar.copy(out_sb[:], co_p[:, P:P + 128])
            cnt = min(TS, NPC - TS * t)
            nc.sync.dma_start(y[TS * t:TS * t + cnt, :], out_sb[0:cnt, :])

    nc.compile()
    return nc


def _expected_src_dst():
    i = np.arange(N)
    offs = np.arange(-WIN, WIN + 1)
    j = i[:, None] + offs[None, :]
    valid = ((j // L) == (i[:, None] // L)) & (j >= 0) & (j < N)
    j = np.where(valid, j, i[:, None])
    dst = np.repeat(i, offs.size).astype(np.int32)
    src = j.reshape(-1).astype(np.int32)
    return src, dst


def _host_inputs(x, pos, ori, W_in, Ws0, bs0, Wk, W_out):
    xf = np.ascontiguousarray(x.reshape(N, C), np.float32)
    pos = np.asarray(pos, np.float32)
    ori = np.asarray(ori, np.float32)
    f16 = np.float16

    # shared weights / constants
    WS = np.zeros((136, K17 * KC), np.float32)
    for k in range(K17):
        s = _sidx(k)
        WS[8 * k:8 * k + 7, KC * k:KC * (k + 1)] = Ws0[s]
        WS[8 * k + 7, KC * k:KC * (k + 1)] = bs0[s]
    # pair-duplicate columns: WS2[:, 48k + 2c + j] = WS[:, 24k + c]
    WS2 = np.repeat(WS, 2, axis=1)
    # delta slot 6 carries raw dist on device; fold the 1/R into the weights
    for k in range(K17):
        WS2[8 * k + 6, :] /= R
    wk_p = np.zeros((128, 6 * W), np.float32)
    for b in range(6):
        wk_p[:, W * b:W * (b + 1)] = Wk[128 * b:128 * (b + 1), :]
    shifts = np.zeros((128, K17 * TS), np.float32)
    for k in range(K17):
        for p in range(TS):
            shifts[p + k, TS * k + p] = 1.0
    common = dict(
        w_in=W_in.astype(f16),
        ws2a=WS2[0:128].astype(f16),
        ws2b=WS2[128:136].astype(f16),
        wk_p=wk_p.astype(f16),
        w_out=W_out.astype(f16),
        ident=np.eye(128, dtype=f16),
        shifts=shifts.astype(f16),
    )

    # self-edge compensation: kself[n] = lrelu(rn @ W5[3:6] + b5, 0.2) * ncl
    rn = (ori.reshape(N, 3, 3) ** 2).sum(axis=2)          # [N, 3]
    pself = rn @ np.asarray(Ws0[S_HALF][3:6], np.float32) \
        + np.asarray(bs0[S_HALF], np.float32)             # [N, KC]
    kself_full = np.where(pself >= 0, pself, NEG_K * pself)

    in_maps = []
    for ci in range(NCORES):
        s0 = ci * NPC
        g = s0 - WIN + np.arange(HR)
        # chain-aware zero padding: out-of-chain halo rows get h = 0, so
        # their messages vanish with no explicit masking on device.
        ok = (g // L) == (s0 // L)
        gi = np.clip(g, 0, N - 1)
        x_pad = np.where(ok[:, None], xf[gi], 0.0).astype(np.float32)
        p_pad = np.where(ok[:, None], pos[gi], 0.0).astype(np.float32)
        o_pad = np.where(ok[:, None], ori[gi], 0.0).astype(np.float32)

        jj, pp = np.meshgrid(np.arange(NT), np.arange(128), indexing="ij")
        rows = (TS * jj + pp)            # [NT, 128] all < HR
        # xT_slot: [128(c), (t, p)] transposed slots
        x_sl = x_pad[rows]               # [NT, 128, C]
        xT_slot = np.ascontiguousarray(
            x_sl.transpose(2, 0, 1).reshape(C, NT * 128)).astype(f16)
        # pos: center per slot for fp16 precision; interleave with ori
        p_sl = p_pad[rows]               # [NT, 128, 3]
        ctr = p_sl.mean(axis=1, keepdims=True)
        pog = np.concatenate([p_sl - ctr, o_pad[rows]], axis=2)  # [NT,128,12]
        pog_slot = np.ascontiguousarray(
            pog.transpose(1, 0, 2).reshape(128, NT * 12)).astype(f16)
        # identity (center rows)
        rc = WIN + TS * jj + pp
        okc = rc < HR
        xc_slot = np.where(okc[:, :, None], x_pad[np.minimum(rc, HR - 1)], 0.0)
        xc_slot = xc_slot.transpose(1, 0, 2).reshape(128, NT * C).astype(f16)

        # boundary-count + kself2 (output-node indexed)
        ncl = np.zeros((128, NT), np.float32)
        for t in (0, NT - 1):
            nvalid = min(TS, NPC - TS * t)
            for p in range(nvalid):
                off = (s0 + TS * t + p) % L
                v = ((off + np.arange(-WIN, WIN + 1)) >= 0) & \
                    ((off + np.arange(-WIN, WIN + 1)) < L)
                ncl[p, t] = K17 - v.sum()
        ks = np.zeros((128, NT, KC), np.float32)
        for t in (0, NT - 1):
            nvalid = min(TS, NPC - TS * t)
            rowsn = s0 + TS * t + np.arange(nvalid)
            ks[:nvalid, t, :] = kself_full[rowsn] * ncl[:nvalid, t][:, None]
        ks2 = np.repeat(ks, 2, axis=2)  # duplicate pairs within each KC block
        in_maps.append(dict(
            xT_slot=xT_slot, xc_slot=xc_slot, pog_slot=pog_slot,
            kself2=ks2.reshape(128, NT * 2 * KC).astype(f16),
            **common))
    return in_maps


def kernel(x, pos, seq, ori, W_in, Ws0, bs0, Wk, W_out, src, dst):
    exp_src, exp_dst = _expected_src_dst()
    assert np.array_equal(np.asarray(src), exp_src), "unexpected src graph"
    assert np.array_equal(np.asarray(dst), exp_dst), "unexpected dst graph"

    from concourse.bass_utils import run_bass_kernel_spmd

    if "nc" not in _PROG:
        _PROG["nc"] = _build_program()
    nc = _PROG["nc"]

    in_maps = _host_inputs(np.asarray(x), np.asarray(pos), np.asarray(ori),
                           np.asarray(W_in), np.asarray(Ws0), np.asarray(bs0),
                           np.asarray(Wk), np.asarray(W_out))
    res = run_bass_kernel_spmd(nc, in_maps, list(range(NCORES)))
    out = np.concatenate([res.results[i]["y"] for i in range(NCORES)], axis=0)
    return out.reshape(B, L, C).astype(np.float32)


# revision 76
# speedup vs baseline: 1.0480x; 1.0281x over previous
"""Bass/Trainium2 kernel for nn_BasicBlock_73933567033945 (CDConv / gnn_message_passing).

v2 strategy (graph = fixed +-8 sequence window inside 4 chains, verified at
runtime): shard 8192 nodes across 8 cores (1024 each, half a chain), slot
layout of 128-row halo windows at stride 112.  All matmuls and DVE tensor ops
run in fp16 (fp32 PSUM accumulation); pos is slot-centered on host so fp16
holds precision.  The 17 window shifts are materialized once per core by 17
wide shift-matmuls over all 10 slots (h|pos|ori, 440 cols each).  The
per-edge kernel MLP output is written pair-duplicated (kern2) so the
bilinear kern (x) h product runs in the DVE 2x perf mode.  The (offset,
channel) contraction runs on the PE via PSUM-accumulated transposes followed
by Wk-chunk matmuls, all fp16.  Pure data parallel: no collectives.
"""
import numpy as np

B, L, C = 4, 2048, 128
N = B * L
W = 32
KC = 24
SEQ_L = 11
R = 12.0
WIN = 8
NEG_IN = 0.1
NEG_K = 0.2
NCORES = 8
NPC = N // NCORES          # 1024 nodes per core
TS = 112                   # output nodes per tile
NT = 10                    # tiles per core (9*112 + 16)
HR = 9 * TS + 128          # 1136 halo rows per core
K17 = 2 * WIN + 1          # 17 window offsets
S_HALF = SEQ_L // 2
PH = 44                    # phys cols per slot: h(32) | pos(3) | ori(9)
NBW = NT * PH              # 440: NB cols per k

_PROG = {}


def _sidx(k):
    return int(np.clip(k - WIN, -S_HALF, S_HALF)) + S_HALF


def _build_program():
    import concourse.tile as tile
    from concourse import mybir, bacc
    from concourse.bass_utils import run_bass_kernel_spmd  # noqa: F401 (import check)
    from contextlib import ExitStack

    f32 = mybir.dt.float32
    f16 = mybir.dt.float16
    AF = mybir.ActivationFunctionType
    OP = mybir.AluOpType
    AX = mybir.AxisListType

    nc = bacc.Bacc("TRN2", target_bir_lowering=False, debug=False)

    def din(name, shape, dt=f16):
        return nc.dram_tensor(name, shape, dt, kind="ExternalInput").ap()

    xT_slot = din("xT_slot", [128, NT * 128])        # x transposed per slot, f16
    xc_slot = din("xc_slot", [128, NT * C])          # identity (center rows) f16
    pog_slot = din("pog_slot", [128, NT * 12])       # centered pos(3) | ori(9)
    w_in = din("w_in", [C, W])
    ws2a = din("ws2a", [128, 2 * K17 * KC])
    ws2b = din("ws2b", [8, 2 * K17 * KC])
    wk_p = din("wk_p", [128, 6 * W])
    w_out = din("w_out", [W, C])
    ident = din("ident", [128, 128])
    shifts = din("shifts", [128, K17 * TS])
    kself2 = din("kself2", [128, NT * 2 * KC])
    y = nc.dram_tensor("y", [NPC, C], f32, kind="ExternalOutput").ap()

    P = TS  # 112 active partitions

    with tile.TileContext(nc) as tc, ExitStack() as ctx:
        pers = ctx.enter_context(tc.tile_pool(name="pers", bufs=1))

        def load(ap_in, shape, tag, dt=f16):
            t = pers.tile(shape, dt, tag=tag)
            nc.sync.dma_start(t[:], ap_in)
            return t

        # DMA order matters: pos/ori + shift matrices + identity feed the
        # NBg matmuls that start first; xc is only needed at tile ends.
        phys_g = pers.tile([128, NT * 12], f16, tag="phys_g")
        nc.sync.dma_start(phys_g[:], pog_slot)
        sh_sb = pers.tile([128, K17 * TS], f16, tag="shifts")
        for c0 in range(0, K17 * TS, 5 * TS):
            c1 = min(c0 + 5 * TS, K17 * TS)
            nc.sync.dma_start(sh_sb[:, c0:c1], shifts[:, c0:c1])
        id_sb = load(ident, [128, 128], "ident")
        w_in_sb = load(w_in, [C, W], "w_in")
        xT_all = pers.tile([128, NT * 128], f16, tag="xT_all")
        nc.sync.dma_start(xT_all[:, 0:5 * 128], xT_slot[:, 0:5 * 128])
        nc.sync.dma_start(xT_all[:, 5 * 128:], xT_slot[:, 5 * 128:])
        ws2a_sb = load(ws2a, [128, 2 * K17 * KC], "ws2a")
        ws2b_sb = load(ws2b, [8, 2 * K17 * KC], "ws2b")
        wk_sb = load(wk_p, [128, 6 * W], "wk")
        w_out_sb = load(w_out, [W, C], "w_out")
        ks2_sb = load(kself2, [128, NT * 2 * KC], "kself2")
        xc_all = load(xc_slot, [128, NT * C], "xc_all")

        # dist = sqrt(d2 + eps): eps = 1e-4 keeps rec = 1/dist <= 100 (fp16
        # safe; self-edges have D = 0 so local = 0 regardless) while real
        # edge distances (>= ~0.5) are perturbed by < 1e-3 relative.
        eps_sb = pers.tile([128, 1], f32, tag="eps")
        nc.vector.memset(eps_sb[:], 1e-4)
        phys_h = pers.tile([128, NT * W], f16, tag="phys_h")

        # ---- NBg: 17 pos/ori shift matmuls (independent of Phase A) ------
        NBg = pers.tile([P, K17 * NT * 12], f16, tag="NBg")
        NBh = pers.tile([P, K17 * NT * W], f16, tag="NBh")
        GW = NT * 12   # 120
        HW_ = NT * W   # 320
        with tc.tile_pool(name="pNBg", bufs=3, space="PSUM") as pNBg:
            for k in range(K17):
                nb_p = pNBg.tile([P, GW], f32, tag="nbg_p")
                nc.tensor.matmul(nb_p[:], sh_sb[:, TS * k:TS * (k + 1)],
                                 phys_g[:], start=True, stop=True)
                dst = NBg[:, GW * k:GW * (k + 1)]
                if k % 2 == 0:
                    nc.vector.tensor_copy(dst, nb_p[:])
                else:
                    nc.scalar.copy(dst, nb_p[:])

        # ---------------- Phase A: h = lrelu(lrelu(x) @ W_in) per slot -----
        with tc.tile_pool(name="pA", bufs=2) as pA, \
             tc.tile_pool(name="pAp", bufs=2, space="PSUM") as pAp:
            for j in range(NT):
                xlT = pA.tile([128, 128], f16, tag="xlT")
                nc.scalar.activation(xlT[:], xT_all[:, 128 * j:128 * (j + 1)],
                                     AF.Prelu, bias=0.0, scale=1.0, alpha=NEG_IN)
                hp = pAp.tile([128, W], f32, tag="hp")
                nc.tensor.matmul(hp[:], xlT[:], w_in_sb[:], start=True, stop=True)
                nc.scalar.activation(phys_h[:, W * j:W * (j + 1)], hp[:],
                                     AF.Prelu, bias=0.0, scale=1.0, alpha=NEG_IN)

        # ---- NBh: 17 h shift matmuls ------------------------------------
        with tc.tile_pool(name="pNBh", bufs=3, space="PSUM") as pNBh:
            for k in range(K17):
                nb_p = pNBh.tile([P, HW_], f32, tag="nbh_p")
                nc.tensor.matmul(nb_p[:], sh_sb[:, TS * k:TS * (k + 1)],
                                 phys_h[:], start=True, stop=True)
                dst = NBh[:, HW_ * k:HW_ * (k + 1)]
                if k % 2 == 0:
                    nc.scalar.copy(dst, nb_p[:])
                else:
                    nc.vector.tensor_copy(dst, nb_p[:])

        # ---------------- Phase B: per output tile ------------------------
        wrk = ctx.enter_context(tc.tile_pool(name="wrk", bufs=4))
        tpool = ctx.enter_context(tc.tile_pool(name="tmp", bufs=8))
        psG = ctx.enter_context(tc.tile_pool(name="psG", bufs=2, space="PSUM"))
        psD = ctx.enter_context(tc.tile_pool(name="psD", bufs=1, space="PSUM"))
        psP = ctx.enter_context(tc.tile_pool(name="psP", bufs=1, space="PSUM"))
        psC = ctx.enter_context(tc.tile_pool(name="psC", bufs=1, space="PSUM"))

        for t in range(NT):
            # k-strided views into NBg for slot t (pos at off 0, ori at 3)
            def kview(off, width):
                v = NBg[:].rearrange("p (k j) -> p k j", j=GW)
                return v[:, :, 12 * t + off:12 * t + off + width]

            gc = GW * 8 + 12 * t
            pos_c = NBg[:, gc:gc + 3]       # [P, 3] center pos
            ori_c = NBg[:, gc + 3:gc + 12]  # [P, 9] center frame

            # ---- geometry -> dav [P, (k,8)] fp16 -------------------------
            D = wrk.tile([P, K17 * 3], f16, tag="D")
            Dv = D[:].rearrange("p (k a) -> p k a", a=3)
            nc.vector.tensor_sub(Dv, kview(0, 3),
                                 pos_c.unsqueeze(1).broadcast_to([P, K17, 3]))
            sq = wrk.tile([P, K17 * 3], f16, tag="sq")
            nc.scalar.activation(sq[:], D[:], AF.Square, bias=0.0, scale=1.0)
            d2 = wrk.tile([P, K17], f32, tag="d2")
            nc.vector.tensor_reduce(d2[:], sq[:].rearrange("p (k a) -> p k a", a=3),
                                    axis=AX.X, op=OP.add)
            dav = wrk.tile([P, K17 * 8], f16, tag="dav")
            davv = dav[:].rearrange("p (k d) -> p k d", d=8)
            # delta slot 6 holds raw dist (WS2 rows for slot 6 are pre-scaled
            # by 1/R on host); the same values feed the reciprocal.
            nc.scalar.activation(davv[:, :, 6], d2[:], AF.Sqrt,
                                 bias=eps_sb[0:P, 0:1], scale=1.0)
            rec = wrk.tile([P, K17], f16, tag="rec")
            with nc.allow_low_precision(reason="fp16 direction scale is ok"):
                nc.vector.reciprocal(rec[:], davv[:, :, 6])
            # local_a = (sum_b Ri[a,b] * D[k,b]) * rec[k]
            lm = wrk.tile([P, K17 * 9], f16, tag="lm")
            lmv = lm[:].rearrange("p (k a b) -> p k a b", a=3, b=3)
            nc.vector.tensor_mul(
                lmv,
                ori_c.rearrange("p (a b) -> p a b", b=3).unsqueeze(1)
                     .broadcast_to([P, K17, 3, 3]),
                D[:].rearrange("p (k b) -> p k b", b=3).unsqueeze(2)
                    .broadcast_to([P, K17, 3, 3]))
            locr = wrk.tile([P, K17 * 3], f16, tag="locr")
            with nc.allow_low_precision(reason="3-term sums, fp16 ok"):
                nc.vector.tensor_reduce(
                    locr[:].rearrange("p (k a) -> p k a", a=3), lmv,
                    axis=AX.X, op=OP.add)
            nc.vector.tensor_mul(
                davv[:, :, 0:3], locr[:].rearrange("p (k a) -> p k a", a=3),
                rec[:].unsqueeze(-1).broadcast_to([P, K17, 3]))
            # ofeat_a = sum_b Ri[a,b] * Rj[a,b]
            ofm = wrk.tile([P, K17 * 9], f16, tag="ofm")
            nc.vector.tensor_mul(
                ofm[:].rearrange("p (k e) -> p k e", e=9), kview(3, 9),
                ori_c.unsqueeze(1).broadcast_to([P, K17, 9]))
            with nc.allow_low_precision(reason="3-term sums, fp16 ok"):
                nc.vector.tensor_reduce(
                    davv[:, :, 3:6],
                    ofm[:].rearrange("p (k a b) -> p k a b", a=3, b=3),
                    axis=AX.X, op=OP.add)
            # out-of-chain halo rows are zero-padded on host, so h_src = 0
            # there and fake-edge messages vanish without any masking.  The
            # dav bias slot (7) is only ever written here, so set it just
            # once per pool buffer rotation.
            if t < 4:
                nc.vector.memset(davv[:, :, 7], 1.0)

            # ---- kern2 = lrelu(dav @ WS2, 0.2), pair-duplicated ----------
            dT_p = psD.tile([128, 224], f16, tag="dT")
            nc.tensor.matmul(dT_p[:, 0:P], dav[:, 0:128], id_sb[0:P, 0:P],
                             is_transpose=True, start=True, stop=False,
                             skip_group_check=True)
            nc.tensor.matmul(dT_p[0:8, P:P + P], dav[:, 128:136], id_sb[0:P, 0:P],
                             is_transpose=True, start=False, stop=True,
                             skip_group_check=True)
            dT = wrk.tile([128, 224], f16, tag="dT_sb")
            nc.scalar.copy(dT[:], dT_p[:])
            W2 = 2 * K17 * KC  # 816
            # psum banks are 512 f32 cols: put k-blocks 0..9 at 0:480 (bank 0)
            # and k-blocks 10..16 at 512:848 (bank 1) to avoid bank crossing.
            pre_p = psP.tile([P, 848], f32, tag="pre")
            nc.tensor.matmul(pre_p[:, 0:480], dT[:, 0:P], ws2a_sb[:, 0:480],
                             start=True, stop=False, skip_group_check=True)
            nc.tensor.matmul(pre_p[:, 512:848], dT[:, 0:P], ws2a_sb[:, 480:W2],
                             start=True, stop=False, skip_group_check=True)
            nc.tensor.matmul(pre_p[:, 0:480], dT[0:8, P:P + P], ws2b_sb[:, 0:480],
                             start=False, stop=True, skip_group_check=True)
            nc.tensor.matmul(pre_p[:, 512:848], dT[0:8, P:P + P], ws2b_sb[:, 480:W2],
                             start=False, stop=True, skip_group_check=True)
            kern2 = wrk.tile([P, W2], f16, tag="kern2")
            nc.scalar.activation(kern2[:, 0:480], pre_p[:, 0:480], AF.Prelu,
                                 bias=0.0, scale=1.0, alpha=NEG_K)
            nc.scalar.activation(kern2[:, 480:W2], pre_p[:, 512:848], AF.Prelu,
                                 bias=0.0, scale=1.0, alpha=NEG_K)
            # self-edge compensation (host-precomputed, pair-duplicated);
            # nonzero only at chain ends, which land in tiles 0 and 9
            if t in (0, NT - 1):
                K8 = 2 * KC * 8
                nc.vector.tensor_add(kern2[:, K8:K8 + 2 * KC],
                                     kern2[:, K8:K8 + 2 * KC],
                                     ks2_sb[0:P, 2 * KC * t:2 * KC * (t + 1)])

            # ---- bilinear + PE transpose-accumulate ----------------------
            # gpsimd takes the last 3 offsets (issued first so they finish
            # by the time the PE transpose chain reaches them); DVE does the
            # rest in the 2x packed mode.
            # sum_k kern_k (x) h_k accumulated in normal layout via
            # identity-stationary copy-matmuls (2 per k, split at the psum
            # bank boundary), then ONE transpose set of 6 matmuls.  The
            # same psum tile is reused for the transposed result after the
            # accumulation has been copied out (WAR dep keeps it safe).
            ag_ps = psG.tile([128, 768], f32, tag="agg")
            agg_p = ag_ps[0:P, :]
            for k in range(K17):
                tm = tpool.tile([P, KC * W], f16, tag="tm")
                hv = NBh[:, HW_ * k + W * t:HW_ * k + W * (t + 1)] \
                    .rearrange("p (s two) -> p s two", two=2) \
                    .unsqueeze(1).broadcast_to([P, KC, 16, 2])
                kv = kern2[:, 2 * KC * k:2 * KC * (k + 1)] \
                    .rearrange("p (c two) -> p c two", two=2) \
                    .unsqueeze(2).broadcast_to([P, KC, 16, 2])
                nc.vector.tensor_tensor(
                    tm[:].rearrange("p (c s two) -> p c s two", two=2, s=16),
                    hv, kv, op=OP.mult)
                nc.tensor.matmul(agg_p[:, 0:512], id_sb[0:P, 0:P],
                                 tm[:, 0:512], start=(k == 0), stop=(k == 16),
                                 skip_group_check=True)
                nc.tensor.matmul(agg_p[:, 512:768], id_sb[0:P, 0:P],
                                 tm[:, 512:768], start=(k == 0), stop=(k == 16),
                                 skip_group_check=True)
            agg = wrk.tile([P, 768], f16, tag="agg_sb")
            nc.scalar.copy(agg[:], agg_p[:])
            aggT_p = ag_ps
            for b in range(6):
                nc.tensor.matmul(aggT_p[:, 128 * b:128 * b + P],
                                 agg[:, 128 * b:128 * (b + 1)], id_sb[0:P, 0:P],
                                 start=(b in (0, 4)), stop=(b in (3, 5)),
                                 skip_group_check=True)
            aggT = wrk.tile([128, 768], f16, tag="aggT_sb")
            nc.scalar.copy(aggT[:, 0:512], aggT_p[:, 0:512])
            nc.scalar.copy(aggT[:, 512:768], aggT_p[:, 512:768])

            # ---- conv = lrelu(agg @ Wk, 0.1) ; out = conv @ W_out + x ----
            co_p = psC.tile([P, 240], f32, tag="co")
            for b in range(6):
                nc.tensor.matmul(co_p[0:W, 0:P], wk_sb[:, W * b:W * (b + 1)],
                                 aggT[:, 128 * b:128 * b + P],
                                 start=(b == 0), stop=(b == 5),
                                 skip_group_check=True)
            convL = wrk.tile([W, P], f16, tag="convL")
            nc.scalar.activation(convL[:], co_p[0:W, 0:P], AF.Prelu, bias=0.0,
                                 scale=1.0, alpha=NEG_IN)
            nc.tensor.matmul(co_p[:, P:P + 128], convL[:], w_out_sb[:],
                             start=True, stop=False, skip_group_check=True)
            # identity add on the PE: accumulate xc into the same psum group
            # via an identity-stationary copy-matmul, then DMA from PSUM.
            nc.tensor.matmul(co_p[:, P:P + 128], id_sb[0:P, 0:P],
                             xc_all[0:P, C * t:C * t + C],
                             start=False, stop=True, skip_group_check=True)
            out_sb = wrk.tile([P, C], f32, tag="out_sb")
            nc.scalar.copy(out_sb[:], co_p[:, P:P + 128])
            cnt = min(TS, NPC - TS * t)
            nc.sync.dma_start(y[TS * t:TS * t + cnt, :], out_sb[0:cnt, :])

    nc.compile()
    return nc


def _expected_src_dst():
    i = np.arange(N)
    offs = np.arange(-WIN, WIN + 1)
    j = i[:, None] + offs[None, :]
    valid = ((j // L) == (i[:, None] // L)) & (j >= 0) & (j < N)
    j = np.where(valid, j, i[:, None])
    dst = np.repeat(i, offs.size).astype(np.int32)
    src = j.reshape(-1).astype(np.int32)
    return src, dst


def _host_inputs(x, pos, ori, W_in, Ws0, bs0, Wk, W_out):
    xf = np.ascontiguousarray(x.reshape(N, C), np.float32)
    pos = np.asarray(pos, np.float32)
    ori = np.asarray(ori, np.float32)
    f16 = np.float16

    # shared weights / constants
    WS = np.zeros((136, K17 * KC), np.float32)
    for k in range(K17):
        s = _sidx(k)
        WS[8 * k:8 * k + 7, KC * k:KC * (k + 1)] = Ws0[s]
        WS[8 * k + 7, KC * k:KC * (k + 1)] = bs0[s]
    # pair-duplicate columns: WS2[:, 48k + 2c + j] = WS[:, 24k + c]
    WS2 = np.repeat(WS, 2, axis=1)
    # delta slot 6 carries raw dist on device; fold the 1/R into the weights
    for k in range(K17):
        WS2[8 * k + 6, :] /= R
    wk_p = np.zeros((128, 6 * W), np.float32)
    for b in range(6):
        wk_p[:, W * b:W * (b + 1)] = Wk[128 * b:128 * (b + 1), :]
    shifts = np.zeros((128, K17 * TS), np.float32)
    for k in range(K17):
        for p in range(TS):
            shifts[p + k, TS * k + p] = 1.0
    common = dict(
        w_in=W_in.astype(f16),
        ws2a=WS2[0:128].astype(f16),
        ws2b=WS2[128:136].astype(f16),
        wk_p=wk_p.astype(f16),
        w_out=W_out.astype(f16),
        ident=np.eye(128, dtype=f16),
        shifts=shifts.astype(f16),
    )

    # self-edge compensation: kself[n] = lrelu(rn @ W5[3:6] + b5, 0.2) * ncl
    rn = (ori.reshape(N, 3, 3) ** 2).sum(axis=2)          # [N, 3]
    pself = rn @ np.asarray(Ws0[S_HALF][3:6], np.float32) \
        + np.asarray(bs0[S_HALF], np.float32)             # [N, KC]
    kself_full = np.where(pself >= 0, pself, NEG_K * pself)

    in_maps = []
    for ci in range(NCORES):
        s0 = ci * NPC
        g = s0 - WIN + np.arange(HR)
        # chain-aware zero padding: out-of-chain halo rows get h = 0, so
        # their messages vanish with no explicit masking on device.
        ok = (g // L) == (s0 // L)
        gi = np.clip(g, 0, N - 1)
        x_pad = np.where(ok[:, None], xf[gi], 0.0).astype(np.float32)
        p_pad = np.where(ok[:, None], pos[gi], 0.0).astype(np.float32)
        o_pad = np.where(ok[:, None], ori[gi], 0.0).astype(np.float32)

        jj, pp = np.meshgrid(np.arange(NT), np.arange(128), indexing="ij")
        rows = (TS * jj + pp)            # [NT, 128] all < HR
        # xT_slot: [128(c), (t, p)] transposed slots
        x_sl = x_pad[rows]               # [NT, 128, C]
        xT_slot = np.ascontiguousarray(
            x_sl.transpose(2, 0, 1).reshape(C, NT * 128)).astype(f16)
        # pos: center per slot for fp16 precision; interleave with ori
        p_sl = p_pad[rows]               # [NT, 128, 3]
        ctr = p_sl.mean(axis=1, keepdims=True)
        pog = np.concatenate([p_sl - ctr, o_pad[rows]], axis=2)  # [NT,128,12]
        pog_slot = np.ascontiguousarray(
            pog.transpose(1, 0, 2).reshape(128, NT * 12)).astype(f16)
        # identity (center rows)
        rc = WIN + TS * jj + pp
        okc = rc < HR
        xc_slot = np.where(okc[:, :, None], x_pad[np.minimum(rc, HR - 1)], 0.0)
        xc_slot = xc_slot.transpose(1, 0, 2).reshape(128, NT * C).astype(f16)

        # boundary-count + kself2 (output-node indexed)
        ncl = np.zeros((128, NT), np.float32)
        for t in (0, NT - 1):
            nvalid = min(TS, NPC - TS * t)
            for p in range(nvalid):
                off = (s0 + TS * t + p) % L
                v = ((off + np.arange(-WIN, WIN + 1)) >= 0) & \
                    ((off + np.arange(-WIN, WIN + 1)) < L)
                ncl[p, t] = K17 - v.sum()
        ks = np.zeros((128, NT, KC), np.float32)
        for t in (0, NT - 1):
            nvalid = min(TS, NPC - TS * t)
            rowsn = s0 + TS * t + np.arange(nvalid)
            ks[:nvalid, t, :] = kself_full[rowsn] * ncl[:nvalid, t][:, None]
        ks2 = np.repeat(ks, 2, axis=2)  # duplicate pairs within each KC block
        in_maps.append(dict(
            xT_slot=xT_slot, xc_slot=xc_slot, pog_slot=pog_slot,
            kself2=ks2.reshape(128, NT * 2 * KC).astype(f16),
            **common))
    return in_maps


def kernel(x, pos, seq, ori, W_in, Ws0, bs0, Wk, W_out, src, dst):
    exp_src, exp_dst = _expected_src_dst()
    assert np.array_equal(np.asarray(src), exp_src), "unexpected src graph"
    assert np.array_equal(np.asarray(dst), exp_dst), "unexpected dst graph"

    from concourse.bass_utils import run_bass_kernel_spmd

    if "nc" not in _PROG:
        _PROG["nc"] = _build_program()
    nc = _PROG["nc"]

    in_maps = _host_inputs(np.asarray(x), np.asarray(pos), np.asarray(ori),
                           np.asarray(W_in), np.asarray(Ws0), np.asarray(bs0),
                           np.asarray(Wk), np.asarray(W_out))
    res = run_bass_kernel_spmd(nc, in_maps, list(range(NCORES)))
    out = np.concatenate([res.results[i]["y"] for i in range(NCORES)], axis=0)
    return out.reshape(B, L, C).astype(np.float32)
